# revision 2
# baseline (speedup 1.0000x reference)
"""GPPT (GCN + prompt MoE routing) Trainium2 kernel v3, 8-core SPMD.

Row-shards N=8192 nodes across 8 cores (1024 rows each). Architecture
per core (validated by end-to-end precision simulation on the exact
harness inputs):

  L0 (sensitive path):  TT = F^T @ adjT_blk  -- 3-pass fp16 hi/lo split
  h0 = relu(W0^T @ TT * 2^-13 + b0)          -- fp32 matmuls
  Y1 = h0 @ W1                               -- fp32 matmuls (score margin)
  AllGather(Y1-hi only, 1MB/rank)
  L1:  h1 = relu(Yh^T @ adjT-hi * 2^-13 + b1) -- 1-PASS fp16 (tolerant:
       score error via this path is ~1e-8 after cross-expert cancellation)
  scores = hc @ Wp                           -- exact fp32 matmuls
  out_all = hc16 @ Wpp16                     -- 1-pass fp16 (values only)
  one-hot argmax select                      -- as baseline

Precision: routing argmax has min top-2 gap 1.05e-7 on this input; only
the L0->h0 path needs fp32-grade accuracy (3-pass + fp32 h0 + fp32
scores). Simulated end-to-end: 0 routing flips, rel err ~1.2e-6.

Schedule: single AllGather of Yh, partially hidden by the h0-half of the
heads matmuls; L1 runs as two n-half sweeps (banks 0-3) with Y k-tiles
resident in SBUF after sweep A; heads partials accumulate in banks 4-7
across the L1 phase.
"""

import os
import numpy as np

import concourse.bass as bass
import concourse.mybir as mybir
import concourse.tile as tile
from concourse import bacc
from concourse.bass_utils import run_bass_kernel_spmd

N = 8192
IN = 512
H = 512
C = 32
E = 7
NCORES = 8
BLK = N // NCORES          # 1024 nodes per core
KT = N // 128              # 64 contraction k-tiles over nodes
SCALE = 8192.0             # adj pre-scale (exact power of two)
NW = E + E * C             # 231

F32 = mybir.dt.float32
F16 = mybir.dt.float16

LAST_RESULTS = None
_CACHED_NC = None


def _kernel_body(ctx, tc, aps):
    nc = tc.nc
    AFT = mybir.ActivationFunctionType
    ALU = mybir.AluOpType

    F_cat = aps["F_cat"]
    A_h, A_l = aps["A_h"], aps["A_l"]
    W0, W1f = aps["W0"], aps["W1f"]
    b0, b1 = aps["b0"], aps["b1"]
    Wp32, Wpp16 = aps["Wp32"], aps["Wpp16"]
    iota7 = aps["iota7"]
    out = aps["out"]
    cc_in, cc_out = aps["cc_in"], aps["cc_out"]

    const = ctx.enter_context(tc.tile_pool(name="const", bufs=1))
    acts = ctx.enter_context(tc.tile_pool(name="acts", bufs=1))
    stream = ctx.enter_context(tc.tile_pool(name="stream", bufs=4))
    ypool = ctx.enter_context(tc.tile_pool(name="ypool", bufs=1))
    small = ctx.enter_context(tc.tile_pool(name="small", bufs=4))
    psum = ctx.enter_context(tc.tile_pool(name="psum", bufs=1, space="PSUM"))

    ps = [psum.tile([128, 512], F32, name=f"bank{i}") for i in range(8)]

    # ---------- L0 stream prefetch for first tiles, then consts ----------
    l0_tiles = []

    def l0_fetch(k):
        fc = stream.tile([128, 1024], F16, name="fc")
        ah = stream.tile([128, 1024], F16, name="ah")
        al = stream.tile([128, 1024], F16, name="al")
        r = slice(k * 128, (k + 1) * 128)
        nc.sync.dma_start(fc[:], F_cat[r, :])
        nc.sync.dma_start(ah[:], A_h[r, :])
        nc.sync.dma_start(al[:], A_l[r, :])
        return fc, ah, al

    # prefetch k=0 before const weights so the PE starts ASAP
    l0_tiles.append(l0_fetch(0))

    w0_t, w1_t, wp_t, wpp_t = [], [], [], []
    for k in range(4):
        t = const.tile([128, H], F32, name=f"w0_{k}")
        nc.sync.dma_start(t[:], W0[k * 128:(k + 1) * 128, :])
        w0_t.append(t)
        t = const.tile([128, H], F32, name=f"w1_{k}")
        nc.sync.dma_start(t[:], W1f[k * 128:(k + 1) * 128, :])
        w1_t.append(t)
    for k in range(8):
        t = const.tile([128, E], F32, name=f"wp_{k}")
        nc.sync.dma_start(t[:], Wp32[k * 128:(k + 1) * 128, :])
        wp_t.append(t)
        t = const.tile([128, E * C], F16, name=f"wpp_{k}")
        nc.sync.dma_start(t[:], Wpp16[k * 128:(k + 1) * 128, :])
        wpp_t.append(t)
    b0_t, b1_t = [], []
    for m in range(4):
        t = const.tile([128, 1], F32, name=f"b0_{m}")
        nc.sync.dma_start(t[:], b0[m * 128:(m + 1) * 128, :])
        b0_t.append(t)
        t = const.tile([128, 1], F32, name=f"b1_{m}")
        nc.sync.dma_start(t[:], b1[m * 128:(m + 1) * 128, :])
        b1_t.append(t)
    iota_t = const.tile([128, E], F32, name="iota7")
    nc.sync.dma_start(iota_t[:], iota7[:, :])

    # ---------------- L0: TT[m,n] = sum_k F[k][:,m].T @ A[k][:,n] --------
    for k in range(KT):
        fc, ah, al = l0_tiles[k]
        if k + 1 < KT:
            l0_tiles.append(l0_fetch(k + 1))
        for p, (lo, rt) in enumerate(((0, ah), (512, ah), (0, al))):
            for m in range(4):
                for n in range(2):
                    nc.tensor.matmul(
                        ps[m * 2 + n][:],
                        fc[:, lo + m * 128:lo + (m + 1) * 128],
                        rt[:, n * 512:(n + 1) * 512],
                        start=(k == 0 and p == 0),
                        stop=(k == KT - 1 and p == 2),
                    )

    # ---------------- TT drain, h0 (fp32), h0h cast ----------------------
    tt = [acts.tile([128, BLK], F32, name=f"tt_{m}") for m in range(4)]
    for m in range(4):
        for n in range(2):
            nc.vector.tensor_copy(tt[m][:, n * 512:(n + 1) * 512],
                                  ps[m * 2 + n][:])

    h0t = [acts.tile([128, BLK], F32, name=f"h0t_{m}") for m in range(4)]
    h0h = [acts.tile([128, BLK], F16, name=f"h0h_{m}") for m in range(4)]
    for m in range(4):
        for n in range(2):
            pt = ps[m * 2 + n]
            for k in range(4):
                nc.tensor.matmul(
                    pt[:],
                    w0_t[k][:, m * 128:(m + 1) * 128],
                    tt[k][:, n * 512:(n + 1) * 512],
                    start=(k == 0),
                    stop=(k == 3),
                )
            nc.scalar.activation(
                h0t[m][:, n * 512:(n + 1) * 512], pt[:],
                AFT.Relu, bias=b0_t[m][:], scale=1.0 / SCALE,
            )
            nc.vector.tensor_copy(h0h[m][:, n * 512:(n + 1) * 512],
                                  h0t[m][:, n * 512:(n + 1) * 512])

    # ---------------- Y1 = h0 @ W1, 1-pass fp16; gather Yh --------------
    yh = [ypool.tile([128, H], F16, name=f"yh_{m}") for m in range(8)]
    for m in range(8):
        pt = ps[m]
        for k in range(4):
            nc.tensor.matmul(
                pt[:],
                h0t[k][:, m * 128:(m + 1) * 128],
                w1_t[k][:],
                start=(k == 0),
                stop=(k == 3),
            )
        nc.vector.tensor_copy(yh[m][:], pt[:])
        nc.sync.dma_start(cc_in[m * 128:(m + 1) * 128, :], yh[m][:])

    nc.gpsimd.collective_compute(
        "AllGather",
        mybir.AluOpType.bypass,
        replica_groups=[list(range(NCORES))],
        ins=[cc_in.opt()],
        outs=[cc_out.opt()],
    )

    # ---------------- heads partials (h0 half) while AG runs -------------
    # heads region for node-block m lives in bank 4 + m//2, col 256*(m%2);
    # scores at cols 0:7, out_all at cols 16:240 (aligned)
    def hreg(m, c0, c1):
        return ps[4 + m // 2][:, 256 * (m % 2) + c0:256 * (m % 2) + c1]

    hpart = [acts.tile([128, 240], F32, name=f"hpart_{m}") for m in range(8)]
    for m in range(8):
        for k in range(4):
            nc.tensor.matmul(
                hreg(m, 0, E),
                h0t[k][:, m * 128:(m + 1) * 128],
                wp_t[4 + k][:],
                start=(k == 0), stop=(k == 3),
            )
        for k in range(4):
            nc.tensor.matmul(
                hreg(m, 16, 16 + E * C),
                h0h[k][:, m * 128:(m + 1) * 128],
                wpp_t[4 + k][:],
                start=(k == 0), stop=(k == 3),
            )
        nc.vector.tensor_copy(hpart[m][:], hreg(m, 0, 240))

    # ---------------- L1: two n-half sweeps, banks 0-3 ------------------
    yk = [ypool.tile([128, H], F16, name=f"yk_{k}") for k in range(KT)]
    h1t = [acts.tile([128, BLK], F32, name=f"h1t_{m}") for m in range(4)]
    h1h = [acts.tile([128, BLK], F16, name=f"h1h_{m}") for m in range(4)]

    for n in range(2):
        for k in range(KT):
            r = slice(k * 128, (k + 1) * 128)
            if n == 0:
                nc.sync.dma_start(yk[k][:], cc_out[r, :])
            ahh = stream.tile([128, 512], F16, name="ahh")
            nc.sync.dma_start(ahh[:], A_h[r, n * 512:(n + 1) * 512])
            for m in range(4):
                nc.tensor.matmul(
                    ps[m][:],
                    yk[k][:, m * 128:(m + 1) * 128],
                    ahh[:],
                    start=(k == 0),
                    stop=(k == KT - 1),
                )
        for m in range(4):
            nc.scalar.activation(
                h1t[m][:, n * 512:(n + 1) * 512], ps[m][:],
                AFT.Relu, bias=b1_t[m][:], scale=1.0 / SCALE,
            )
            nc.vector.tensor_copy(h1h[m][:, n * 512:(n + 1) * 512],
                                  h1t[m][:, n * 512:(n + 1) * 512])

    # ---------------- heads (h1 half) + routing select -------------------
    for m in range(8):
        for k in range(4):
            nc.tensor.matmul(
                hreg(m, 0, E),
                h1t[k][:, m * 128:(m + 1) * 128],
                wp_t[k][:],
                start=(k == 0), stop=(k == 3),
            )
        for k in range(4):
            nc.tensor.matmul(
                hreg(m, 16, 16 + E * C),
                h1h[k][:, m * 128:(m + 1) * 128],
                wpp_t[k][:],
                start=(k == 0), stop=(k == 3),
            )
        hfin = small.tile([128, 240], F32, name="hfin")
        nc.vector.tensor_tensor(hfin[:], hreg(m, 0, 240), hpart[m][:],
                                op=ALU.add)
        sc = hfin[:, 0:E]
        oa = hfin[:, 16:16 + E * C]
        rmax = small.tile([128, 1], F32, name="rmax")
        nc.vector.tensor_reduce(rmax[:], sc, axis=mybir.AxisListType.X, op=ALU.max)
        val = small.tile([128, E], F32, name="val")
        nc.vector.tensor_scalar(val[:], sc, rmax[:], 1024.0, ALU.is_lt, ALU.mult)
        nc.vector.tensor_tensor(val[:], val[:], iota_t[:], op=ALU.add)
        idxf = small.tile([128, 1], F32, name="idxf")
        nc.vector.tensor_reduce(idxf[:], val[:], axis=mybir.AxisListType.X, op=ALU.min)
        onehot = small.tile([128, E], F32, name="onehot")
        nc.vector.tensor_scalar(onehot[:], val[:], idxf[:], None, ALU.is_equal)
        masked = small.tile([128, E, C], F32, name="masked")
        oa_v = oa.rearrange("p (e c) -> p e c", e=E)
        oh_v = onehot[:, :, None].broadcast_to((128, E, C))
        nc.vector.tensor_tensor(masked[:], oa_v, oh_v, op=ALU.mult)
        out_m = small.tile([128, C], F32, name="out_m")
        mv = masked[:].rearrange("p e c -> p c e")
        nc.vector.tensor_reduce(out_m[:], mv, axis=mybir.AxisListType.X, op=ALU.add)
        nc.sync.dma_start(out[m * 128:(m + 1) * 128, :], out_m[:])


def _build_nc():
    nc = bacc.Bacc("TRN2", target_bir_lowering=False, debug=False,
                   num_devices=NCORES)
    aps = {}
    def inp(name, shape, dt):
        aps[name] = nc.dram_tensor(name, shape, dt, kind="ExternalInput").ap()
    inp("F_cat", [N, 2 * IN], F16)
    inp("A_h", [N, BLK], F16)
    inp("A_l", [N, BLK], F16)
    inp("W0", [IN, H], F32)
    inp("W1f", [H, H], F32)
    inp("b0", [H, 1], F32)
    inp("b1", [H, 1], F32)
    inp("Wp32", [2 * H, E], F32)
    inp("Wpp16", [2 * H, E * C], F16)
    inp("iota7", [128, E], F32)
    aps["out"] = nc.dram_tensor("out", [BLK, C], F32, kind="ExternalOutput").ap()
    aps["cc_in"] = nc.dram_tensor("cc_in", [BLK, H], F16).ap()
    aps["cc_out"] = nc.dram_tensor("cc_out", [N, H], F16,
                                   addr_space="Shared").ap()
    from contextlib import ExitStack
    with tile.TileContext(nc) as tc, ExitStack() as ctx:
        _kernel_body(ctx, tc, aps)
    nc.compile()
    return nc


def _split16(x):
    h = x.astype(np.float16)
    l = (x - h.astype(np.float32)).astype(np.float16)
    return h, l


def kernel(feature, adj, W0, b0, W1, b1, Wp, Wpp):
    global LAST_RESULTS, _CACHED_NC
    feature = np.ascontiguousarray(np.asarray(feature, dtype=np.float32))
    adj = np.asarray(adj, dtype=np.float32)
    W0 = np.asarray(W0, dtype=np.float32)
    b0 = np.asarray(b0, dtype=np.float32)
    W1 = np.asarray(W1, dtype=np.float32)
    b1 = np.asarray(b1, dtype=np.float32)
    Wp = np.asarray(Wp, dtype=np.float32)
    Wpp = np.asarray(Wpp, dtype=np.float32)

    if _CACHED_NC is None:
        _CACHED_NC = _build_nc()
    nc = _CACHED_NC

    F_h, F_l = _split16(feature)
    F_cat = np.ascontiguousarray(np.concatenate([F_h, F_l], axis=1))
    Wpp16 = np.ascontiguousarray(
        Wpp.transpose(1, 0, 2).reshape(2 * H, E * C).astype(np.float16))
    iota7 = np.tile(np.arange(E, dtype=np.float32), (128, 1))
    shared = {
        "F_cat": F_cat,
        "W0": np.ascontiguousarray(W0),
        "W1f": np.ascontiguousarray(W1),
        "b0": b0.reshape(H, 1), "b1": b1.reshape(H, 1),
        "Wp32": np.ascontiguousarray(Wp),
        "Wpp16": Wpp16, "iota7": iota7,
    }
    in_maps = []
    for c in range(NCORES):
        blk = adj[c * BLK:(c + 1) * BLK, :].T.astype(np.float32) * SCALE
        A_h, A_l = _split16(blk)
        m = dict(shared)
        m["A_h"] = np.ascontiguousarray(A_h)
        m["A_l"] = np.ascontiguousarray(A_l)
        in_maps.append(m)

    trace = os.environ.get("BASS_KERNEL_TRACE", "0") == "1"
    res = run_bass_kernel_spmd(nc, in_maps, list(range(NCORES)), trace=trace)
    LAST_RESULTS = res
    out = np.concatenate([res.results[c]["out"] for c in range(NCORES)], axis=0)
    return out


# revision 7
# speedup vs baseline: 1.0930x; 1.0930x over previous
"""GPPT (GCN + prompt MoE routing) Trainium2 kernel v3, 8-core SPMD.

Row-shards N=8192 nodes across 8 cores (1024 rows each). Architecture
per core (validated by end-to-end precision simulation on the exact
harness inputs):

  L0 (sensitive path):  TT = F^T @ adjT_blk  -- 3-pass fp16 hi/lo split
  h0 = relu(W0^T @ TT * 2^-13 + b0)          -- fp32 matmuls
  Y1 = h0 @ W1                               -- fp32 matmuls (score margin)
  AllGather(Y1-hi only, 1MB/rank)
  L1:  h1 = relu(Yh^T @ adjT-hi * 2^-13 + b1) -- 1-PASS fp16 (tolerant:
       score error via this path is ~1e-8 after cross-expert cancellation)
  scores = hc @ Wp                           -- exact fp32 matmuls
  out_all = hc16 @ Wpp16                     -- 1-pass fp16 (values only)
  one-hot argmax select                      -- as baseline

Precision: routing argmax has min top-2 gap 1.05e-7 on this input; only
the L0->h0 path needs fp32-grade accuracy (3-pass + fp32 h0 + fp32
scores). Simulated end-to-end: 0 routing flips, rel err ~1.2e-6.

Schedule: single AllGather of Yh, partially hidden by the h0-half of the
heads matmuls; L1 runs as two n-half sweeps (banks 0-3) with Y k-tiles
resident in SBUF after sweep A; heads partials accumulate in banks 4-7
across the L1 phase.
"""

import os
import numpy as np

import concourse.bass as bass
import concourse.mybir as mybir
import concourse.tile as tile
from concourse import bacc
from concourse.bass_utils import run_bass_kernel_spmd

N = 8192
IN = 512
H = 512
C = 32
E = 7
NCORES = 8
BLK = N // NCORES          # 1024 nodes per core
KT = N // 128              # 64 contraction k-tiles over nodes
SCALE = 8192.0             # adj pre-scale (exact power of two)
NW = E + E * C             # 231

F32 = mybir.dt.float32
F16 = mybir.dt.float16

LAST_RESULTS = None
_CACHED_NC = None


def _kernel_body(ctx, tc, aps):
    nc = tc.nc
    AFT = mybir.ActivationFunctionType
    ALU = mybir.AluOpType

    F_cat = aps["F_cat"]
    A_h, A_l = aps["A_h"], aps["A_l"]
    W0, W1f = aps["W0"], aps["W1f"]
    b0, b1 = aps["b0"], aps["b1"]
    Wp32, Wpp16 = aps["Wp32"], aps["Wpp16"]
    iota7 = aps["iota7"]
    out = aps["out"]
    cc_in, cc_out = aps["cc_in"], aps["cc_out"]

    const = ctx.enter_context(tc.tile_pool(name="const", bufs=1))
    acts = ctx.enter_context(tc.tile_pool(name="acts", bufs=1))
    stream = ctx.enter_context(tc.tile_pool(name="stream", bufs=4))
    ypool = ctx.enter_context(tc.tile_pool(name="ypool", bufs=1))
    small = ctx.enter_context(tc.tile_pool(name="small", bufs=4))
    psum = ctx.enter_context(tc.tile_pool(name="psum", bufs=1, space="PSUM"))

    ps = [psum.tile([128, 512], F32, name=f"bank{i}") for i in range(8)]

    # ---------- L0 stream prefetch for first tiles, then consts ----------
    l0_tiles = []

    def l0_fetch(k):
        fc = stream.tile([128, 1024], F16, name="fc")
        ah = stream.tile([128, 1024], F16, name="ah")
        al = stream.tile([128, 1024], F16, name="al")
        r = slice(k * 128, (k + 1) * 128)
        nc.sync.dma_start(fc[:], F_cat[r, :])
        nc.sync.dma_start(ah[:], A_h[r, :])
        nc.sync.dma_start(al[:], A_l[r, :])
        return fc, ah, al

    # prefetch k=0..2 before const weights so the PE starts ASAP and the
    # stream stays ahead of the ~6.3us/tile PE consumption rate; slot the
    # consts (needed only ~400us in) behind the early stream tiles
    for k in range(3):
        l0_tiles.append(l0_fetch(k))

    w0_t, w1_t, wp_t, wpp_t = [], [], [], []
    for k in range(4):
        t = const.tile([128, H], F32, name=f"w0_{k}")
        nc.sync.dma_start(t[:], W0[k * 128:(k + 1) * 128, :])
        w0_t.append(t)
        t = const.tile([128, H], F32, name=f"w1_{k}")
        nc.sync.dma_start(t[:], W1f[k * 128:(k + 1) * 128, :])
        w1_t.append(t)
    l0_tiles.append(l0_fetch(3))
    for k in range(8):
        t = const.tile([128, E], F32, name=f"wp_{k}")
        nc.sync.dma_start(t[:], Wp32[k * 128:(k + 1) * 128, :])
        wp_t.append(t)
        t = const.tile([128, E * C], F16, name=f"wpp_{k}")
        nc.sync.dma_start(t[:], Wpp16[k * 128:(k + 1) * 128, :])
        wpp_t.append(t)
    b0_t, b1_t = [], []
    for m in range(4):
        t = const.tile([128, 1], F32, name=f"b0_{m}")
        nc.sync.dma_start(t[:], b0[m * 128:(m + 1) * 128, :])
        b0_t.append(t)
        t = const.tile([128, 1], F32, name=f"b1_{m}")
        nc.sync.dma_start(t[:], b1[m * 128:(m + 1) * 128, :])
        b1_t.append(t)
    iota_t = const.tile([128, E], F32, name="iota7")
    nc.sync.dma_start(iota_t[:], iota7[:, :])

    # ---------------- L0: TT[m,n] = sum_k F[k][:,m].T @ A[k][:,n] --------
    for k in range(KT):
        fc, ah, al = l0_tiles[k]
        if k + 4 < KT:
            l0_tiles.append(l0_fetch(k + 4))
        for p, (lo, rt) in enumerate(((0, ah), (512, ah), (0, al))):
            for m in range(4):
                for n in range(2):
                    nc.tensor.matmul(
                        ps[m * 2 + n][:],
                        fc[:, lo + m * 128:lo + (m + 1) * 128],
                        rt[:, n * 512:(n + 1) * 512],
                        start=(k == 0 and p == 0),
                        stop=(k == KT - 1 and p == 2),
                    )

    # ---------------- TT drain, h0 (fp32), h0h cast ----------------------
    tt = [acts.tile([128, BLK], F32, name=f"tt_{m}") for m in range(4)]
    for n in range(2):
        for m in range(4):
            nc.vector.tensor_copy(tt[m][:, n * 512:(n + 1) * 512],
                                  ps[m * 2 + n][:])

    h0t = [acts.tile([128, BLK], F32, name=f"h0t_{m}") for m in range(4)]
    h0h = [acts.tile([128, BLK], F16, name=f"h0h_{m}") for m in range(4)]
    for n in range(2):
        for m in range(4):
            pt = ps[m * 2 + n]
            for k in range(4):
                nc.tensor.matmul(
                    pt[:],
                    w0_t[k][:, m * 128:(m + 1) * 128],
                    tt[k][:, n * 512:(n + 1) * 512],
                    start=(k == 0),
                    stop=(k == 3),
                )
            nc.scalar.activation(
                h0t[m][:, n * 512:(n + 1) * 512], pt[:],
                AFT.Relu, bias=b0_t[m][:], scale=1.0 / SCALE,
            )
            nc.vector.tensor_copy(h0h[m][:, n * 512:(n + 1) * 512],
                                  h0t[m][:, n * 512:(n + 1) * 512])

    # ---------------- Y1 = h0 @ W1, 1-pass fp16; gather Yh --------------
    yh = [ypool.tile([128, H], F16, name=f"yh_{m}") for m in range(8)]
    for m in range(8):
        pt = ps[m]
        for k in range(4):
            nc.tensor.matmul(
                pt[:],
                h0t[k][:, m * 128:(m + 1) * 128],
                w1_t[k][:],
                start=(k == 0),
                stop=(k == 3),
            )
        nc.vector.tensor_copy(yh[m][:], pt[:])
        nc.sync.dma_start(cc_in[m * 128:(m + 1) * 128, :], yh[m][:])

    nc.gpsimd.collective_compute(
        "AllGather",
        mybir.AluOpType.bypass,
        replica_groups=[list(range(NCORES))],
        ins=[cc_in.opt()],
        outs=[cc_out.opt()],
    )

    # ---------------- heads partials (h0 half) while AG runs -------------
    # heads region for node-block m lives in bank 4 + m//2, col 256*(m%2);
    # scores at cols 0:7, out_all at cols 16:240 (aligned)
    def hreg(m, c0, c1):
        return ps[4 + m // 2][:, 256 * (m % 2) + c0:256 * (m % 2) + c1]

    # interleave m across banks (bank = 4 + m//2) so one block's PSUM
    # drain never stalls the next block's matmuls (tile-granular deps)
    M_ORDER = (0, 2, 4, 6, 1, 3, 5, 7)

    hpart = [acts.tile([128, 240], F32, name=f"hpart_{m}") for m in range(8)]
    for m in M_ORDER:
        for k in range(4):
            nc.tensor.matmul(
                hreg(m, 0, E),
                h0t[k][:, m * 128:(m + 1) * 128],
                wp_t[4 + k][:],
                start=(k == 0), stop=(k == 3),
            )
        for k in range(4):
            nc.tensor.matmul(
                hreg(m, 16, 16 + E * C),
                h0h[k][:, m * 128:(m + 1) * 128],
                wpp_t[4 + k][:],
                start=(k == 0), stop=(k == 3),
            )
        nc.vector.tensor_copy(hpart[m][:], hreg(m, 0, 240))

    # ---------------- L1: two n-half sweeps, banks 0-3 ------------------
    yk = [ypool.tile([128, H], F16, name=f"yk_{k}") for k in range(KT)]
    h1t = [acts.tile([128, BLK], F32, name=f"h1t_{m}") for m in range(4)]
    h1h = [acts.tile([128, BLK], F16, name=f"h1h_{m}") for m in range(4)]

    for n in range(2):
        for k in range(KT):
            r = slice(k * 128, (k + 1) * 128)
            if n == 0:
                nc.sync.dma_start(yk[k][:], cc_out[r, :])
            ahh = stream.tile([128, 512], F16, name="ahh")
            nc.sync.dma_start(ahh[:], A_h[r, n * 512:(n + 1) * 512])
            for m in range(4):
                nc.tensor.matmul(
                    ps[m][:],
                    yk[k][:, m * 128:(m + 1) * 128],
                    ahh[:],
                    start=(k == 0),
                    stop=(k == KT - 1),
                )
        for m in range(4):
            nc.scalar.activation(
                h1t[m][:, n * 512:(n + 1) * 512], ps[m][:],
                AFT.Relu, bias=b1_t[m][:], scale=1.0 / SCALE,
            )
            nc.vector.tensor_copy(h1h[m][:, n * 512:(n + 1) * 512],
                                  h1t[m][:, n * 512:(n + 1) * 512])

    # ---------------- heads (h1 half) + routing select -------------------
    for m in M_ORDER:
        for k in range(4):
            nc.tensor.matmul(
                hreg(m, 0, E),
                h1t[k][:, m * 128:(m + 1) * 128],
                wp_t[k][:],
                start=(k == 0), stop=(k == 3),
            )
        for k in range(4):
            nc.tensor.matmul(
                hreg(m, 16, 16 + E * C),
                h1h[k][:, m * 128:(m + 1) * 128],
                wpp_t[k][:],
                start=(k == 0), stop=(k == 3),
            )
        hfin = small.tile([128, 240], F32, name="hfin")
        nc.vector.tensor_tensor(hfin[:], hreg(m, 0, 240), hpart[m][:],
                                op=ALU.add)
        sc = hfin[:, 0:E]
        oa = hfin[:, 16:16 + E * C]
        rmax = small.tile([128, 1], F32, name="rmax")
        nc.vector.tensor_reduce(rmax[:], sc, axis=mybir.AxisListType.X, op=ALU.max)
        val = small.tile([128, E], F32, name="val")
        nc.vector.tensor_scalar(val[:], sc, rmax[:], 1024.0, ALU.is_lt, ALU.mult)
        nc.vector.tensor_tensor(val[:], val[:], iota_t[:], op=ALU.add)
        idxf = small.tile([128, 1], F32, name="idxf")
        nc.vector.tensor_reduce(idxf[:], val[:], axis=mybir.AxisListType.X, op=ALU.min)
        onehot = small.tile([128, E], F32, name="onehot")
        nc.vector.tensor_scalar(onehot[:], val[:], idxf[:], None, ALU.is_equal)
        masked = small.tile([128, E, C], F32, name="masked")
        oa_v = oa.rearrange("p (e c) -> p e c", e=E)
        oh_v = onehot[:, :, None].broadcast_to((128, E, C))
        nc.vector.tensor_tensor(masked[:], oa_v, oh_v, op=ALU.mult)
        out_m = small.tile([128, C], F32, name="out_m")
        mv = masked[:].rearrange("p e c -> p c e")
        nc.vector.tensor_reduce(out_m[:], mv, axis=mybir.AxisListType.X, op=ALU.add)
        nc.sync.dma_start(out[m * 128:(m + 1) * 128, :], out_m[:])


def _build_nc():
    nc = bacc.Bacc("TRN2", target_bir_lowering=False, debug=False,
                   num_devices=NCORES)
    aps = {}
    def inp(name, shape, dt):
        aps[name] = nc.dram_tensor(name, shape, dt, kind="ExternalInput").ap()
    inp("F_cat", [N, 2 * IN], F16)
    inp("A_h", [N, BLK], F16)
    inp("A_l", [N, BLK], F16)
    inp("W0", [IN, H], F32)
    inp("W1f", [H, H], F32)
    inp("b0", [H, 1], F32)
    inp("b1", [H, 1], F32)
    inp("Wp32", [2 * H, E], F32)
    inp("Wpp16", [2 * H, E * C], F16)
    inp("iota7", [128, E], F32)
    aps["out"] = nc.dram_tensor("out", [BLK, C], F32, kind="ExternalOutput").ap()
    aps["cc_in"] = nc.dram_tensor("cc_in", [BLK, H], F16).ap()
    aps["cc_out"] = nc.dram_tensor("cc_out", [N, H], F16,
                                   addr_space="Shared").ap()
    from contextlib import ExitStack
    with tile.TileContext(nc) as tc, ExitStack() as ctx:
        _kernel_body(ctx, tc, aps)
    nc.compile()
    return nc


def _split16(x):
    h = x.astype(np.float16)
    l = (x - h.astype(np.float32)).astype(np.float16)
    return h, l


def kernel(feature, adj, W0, b0, W1, b1, Wp, Wpp):
    global LAST_RESULTS, _CACHED_NC
    feature = np.ascontiguousarray(np.asarray(feature, dtype=np.float32))
    adj = np.asarray(adj, dtype=np.float32)
    W0 = np.asarray(W0, dtype=np.float32)
    b0 = np.asarray(b0, dtype=np.float32)
    W1 = np.asarray(W1, dtype=np.float32)
    b1 = np.asarray(b1, dtype=np.float32)
    Wp = np.asarray(Wp, dtype=np.float32)
    Wpp = np.asarray(Wpp, dtype=np.float32)

    if _CACHED_NC is None:
        _CACHED_NC = _build_nc()
    nc = _CACHED_NC

    F_h, F_l = _split16(feature)
    F_cat = np.ascontiguousarray(np.concatenate([F_h, F_l], axis=1))
    Wpp16 = np.ascontiguousarray(
        Wpp.transpose(1, 0, 2).reshape(2 * H, E * C).astype(np.float16))
    iota7 = np.tile(np.arange(E, dtype=np.float32), (128, 1))
    shared = {
        "F_cat": F_cat,
        "W0": np.ascontiguousarray(W0),
        "W1f": np.ascontiguousarray(W1),
        "b0": b0.reshape(H, 1), "b1": b1.reshape(H, 1),
        "Wp32": np.ascontiguousarray(Wp),
        "Wpp16": Wpp16, "iota7": iota7,
    }
    in_maps = []
    for c in range(NCORES):
        blk = adj[c * BLK:(c + 1) * BLK, :].T.astype(np.float32) * SCALE
        A_h, A_l = _split16(blk)
        m = dict(shared)
        m["A_h"] = np.ascontiguousarray(A_h)
        m["A_l"] = np.ascontiguousarray(A_l)
        in_maps.append(m)

    trace = os.environ.get("BASS_KERNEL_TRACE", "0") == "1"
    res = run_bass_kernel_spmd(nc, in_maps, list(range(NCORES)), trace=trace)
    LAST_RESULTS = res
    out = np.concatenate([res.results[c]["out"] for c in range(NCORES)], axis=0)
    return out


# revision 8
# speedup vs baseline: 1.1490x; 1.0512x over previous
"""GPPT (GCN + prompt MoE routing) Trainium2 kernel v3, 8-core SPMD.

Row-shards N=8192 nodes across 8 cores (1024 rows each). Architecture
per core (validated by end-to-end precision simulation on the exact
harness inputs):

  L0 (sensitive path):  TT = F^T @ adjT_blk  -- 3-pass fp16 hi/lo split
  h0 = relu(W0^T @ TT * 2^-13 + b0)          -- fp32 matmuls
  Y1 = h0 @ W1                               -- fp32 matmuls (score margin)
  AllGather(Y1-hi only, 1MB/rank)
  L1:  h1 = relu(Yh^T @ adjT-hi * 2^-13 + b1) -- 1-PASS fp16 (tolerant:
       score error via this path is ~1e-8 after cross-expert cancellation)
  scores = hc @ Wp                           -- exact fp32 matmuls
  out_all = hc16 @ Wpp16                     -- 1-pass fp16 (values only)
  one-hot argmax select                      -- as baseline

Precision: routing argmax has min top-2 gap 1.05e-7 on this input; only
the L0->h0 path needs fp32-grade accuracy (3-pass + fp32 h0 + fp32
scores). Simulated end-to-end: 0 routing flips, rel err ~1.2e-6.

Schedule: single AllGather of Yh, partially hidden by the h0-half of the
heads matmuls; L1 runs as two n-half sweeps (banks 0-3) with Y k-tiles
resident in SBUF after sweep A; heads partials accumulate in banks 4-7
across the L1 phase.
"""

import os
import numpy as np

import concourse.bass as bass
import concourse.mybir as mybir
import concourse.tile as tile
from concourse import bacc
from concourse.bass_utils import run_bass_kernel_spmd

N = 8192
IN = 512
H = 512
C = 32
E = 7
NCORES = 8
BLK = N // NCORES          # 1024 nodes per core
KT = N // 128              # 64 contraction k-tiles over nodes
SCALE = 8192.0             # adj pre-scale (exact power of two)
NW = E + E * C             # 231

F32 = mybir.dt.float32
F16 = mybir.dt.float16

LAST_RESULTS = None
_CACHED_NC = None


def _kernel_body(ctx, tc, aps):
    nc = tc.nc
    AFT = mybir.ActivationFunctionType
    ALU = mybir.AluOpType

    F_cat = aps["F_cat"]
    A_h, A_l = aps["A_h"], aps["A_l"]
    W0, W1f = aps["W0"], aps["W1f"]
    b0, b1 = aps["b0"], aps["b1"]
    Wp32, Wpp16 = aps["Wp32"], aps["Wpp16"]
    iota7 = aps["iota7"]
    out = aps["out"]
    cc_in, cc_out = aps["cc_in"], aps["cc_out"]

    const = ctx.enter_context(tc.tile_pool(name="const", bufs=1))
    acts = ctx.enter_context(tc.tile_pool(name="acts", bufs=1))
    stream = ctx.enter_context(tc.tile_pool(name="stream", bufs=4))
    ypool = ctx.enter_context(tc.tile_pool(name="ypool", bufs=1))
    small = ctx.enter_context(tc.tile_pool(name="small", bufs=4))
    psum = ctx.enter_context(tc.tile_pool(name="psum", bufs=1, space="PSUM"))

    ps = [psum.tile([128, 512], F32, name=f"bank{i}") for i in range(8)]

    # ---------- L0 stream prefetch for first tiles, then consts ----------
    l0_tiles = []

    def l0_fetch(k):
        fc = stream.tile([128, 1024], F16, name="fc")
        ah = stream.tile([128, 1024], F16, name="ah")
        al = stream.tile([128, 1024], F16, name="al")
        r = slice(k * 128, (k + 1) * 128)
        nc.sync.dma_start(fc[:], F_cat[r, :])
        nc.sync.dma_start(ah[:], A_h[r, :])
        nc.sync.dma_start(al[:], A_l[r, :])
        return fc, ah, al

    # prefetch k=0 before const weights so the PE starts ASAP
    l0_tiles.append(l0_fetch(0))

    w0_t, w1_t, wp_t, wpp_t = [], [], [], []
    for k in range(4):
        t = const.tile([128, H], F32, name=f"w0_{k}")
        nc.sync.dma_start(t[:], W0[k * 128:(k + 1) * 128, :])
        w0_t.append(t)
        t = const.tile([128, H], F32, name=f"w1_{k}")
        nc.sync.dma_start(t[:], W1f[k * 128:(k + 1) * 128, :])
        w1_t.append(t)
    for k in range(8):
        t = const.tile([128, E], F32, name=f"wp_{k}")
        nc.sync.dma_start(t[:], Wp32[k * 128:(k + 1) * 128, :])
        wp_t.append(t)
        t = const.tile([128, E * C], F16, name=f"wpp_{k}")
        nc.sync.dma_start(t[:], Wpp16[k * 128:(k + 1) * 128, :])
        wpp_t.append(t)
    b0_t, b1_t = [], []
    for m in range(4):
        t = const.tile([128, 1], F32, name=f"b0_{m}")
        nc.sync.dma_start(t[:], b0[m * 128:(m + 1) * 128, :])
        b0_t.append(t)
        t = const.tile([128, 1], F32, name=f"b1_{m}")
        nc.sync.dma_start(t[:], b1[m * 128:(m + 1) * 128, :])
        b1_t.append(t)
    iota_t = const.tile([128, E], F32, name="iota7")
    nc.sync.dma_start(iota_t[:], iota7[:, :])

    # ---------------- L0: TT[m,n] = sum_k F[k][:,m].T @ A[k][:,n] --------
    for k in range(KT):
        fc, ah, al = l0_tiles[k]
        if k + 1 < KT:
            l0_tiles.append(l0_fetch(k + 1))
        for p, (lo, rt) in enumerate(((0, ah), (512, ah), (0, al))):
            for m in range(4):
                for n in range(2):
                    nc.tensor.matmul(
                        ps[m * 2 + n][:],
                        fc[:, lo + m * 128:lo + (m + 1) * 128],
                        rt[:, n * 512:(n + 1) * 512],
                        start=(k == 0 and p == 0),
                        stop=(k == KT - 1 and p == 2),
                    )

    # ---------------- TT drain, h0 (fp32), h0h cast ----------------------
    tt = [acts.tile([128, BLK], F32, name=f"tt_{m}") for m in range(4)]
    for m in range(4):
        for n in range(2):
            nc.vector.tensor_copy(tt[m][:, n * 512:(n + 1) * 512],
                                  ps[m * 2 + n][:])

    h0t = [acts.tile([128, BLK], F32, name=f"h0t_{m}") for m in range(4)]
    h0h = [acts.tile([128, BLK], F16, name=f"h0h_{m}") for m in range(4)]
    for m in range(4):
        for n in range(2):
            pt = ps[m * 2 + n]
            for k in range(4):
                nc.tensor.matmul(
                    pt[:],
                    w0_t[k][:, m * 128:(m + 1) * 128],
                    tt[k][:, n * 512:(n + 1) * 512],
                    start=(k == 0),
                    stop=(k == 3),
                )
            nc.scalar.activation(
                h0t[m][:, n * 512:(n + 1) * 512], pt[:],
                AFT.Relu, bias=b0_t[m][:], scale=1.0 / SCALE,
            )
            nc.vector.tensor_copy(h0h[m][:, n * 512:(n + 1) * 512],
                                  h0t[m][:, n * 512:(n + 1) * 512])

    # ---------------- Y1 = h0 @ W1, 1-pass fp16; gather Yh --------------
    yh = [ypool.tile([128, H], F16, name=f"yh_{m}") for m in range(8)]
    for m in range(8):
        pt = ps[m]
        for k in range(4):
            nc.tensor.matmul(
                pt[:],
                h0t[k][:, m * 128:(m + 1) * 128],
                w1_t[k][:],
                start=(k == 0),
                stop=(k == 3),
            )
        nc.vector.tensor_copy(yh[m][:], pt[:])
        nc.sync.dma_start(cc_in[m * 128:(m + 1) * 128, :], yh[m][:])

    nc.gpsimd.collective_compute(
        "AllGather",
        mybir.AluOpType.bypass,
        replica_groups=[list(range(NCORES))],
        ins=[cc_in.opt()],
        outs=[cc_out.opt()],
    )

    # ---------------- heads partials (h0 half) while AG runs -------------
    # heads region for node-block m lives in bank 4 + m//2, col 256*(m%2);
    # scores at cols 0:7, out_all at cols 16:240 (aligned)
    def hreg(m, c0, c1):
        return ps[4 + m // 2][:, 256 * (m % 2) + c0:256 * (m % 2) + c1]

    hpart = [acts.tile([128, 240], F32, name=f"hpart_{m}") for m in range(8)]
    for m in range(8):
        for k in range(4):
            nc.tensor.matmul(
                hreg(m, 0, E),
                h0t[k][:, m * 128:(m + 1) * 128],
                wp_t[4 + k][:],
                start=(k == 0), stop=(k == 3),
            )
        for k in range(4):
            nc.tensor.matmul(
                hreg(m, 16, 16 + E * C),
                h0h[k][:, m * 128:(m + 1) * 128],
                wpp_t[4 + k][:],
                start=(k == 0), stop=(k == 3),
            )
        nc.vector.tensor_copy(hpart[m][:], hreg(m, 0, 240))

    # ---------------- L1: two n-half sweeps, banks 0-3 ------------------
    yk = [ypool.tile([128, H], F16, name=f"yk_{k}") for k in range(KT)]
    h1t = [acts.tile([128, BLK], F32, name=f"h1t_{m}") for m in range(4)]
    h1h = [acts.tile([128, BLK], F16, name=f"h1h_{m}") for m in range(4)]

    for n in range(2):
        for k in range(KT):
            r = slice(k * 128, (k + 1) * 128)
            if n == 0:
                nc.sync.dma_start(yk[k][:], cc_out[r, :])
            ahh = stream.tile([128, 512], F16, name="ahh")
            nc.sync.dma_start(ahh[:], A_h[r, n * 512:(n + 1) * 512])
            for m in range(4):
                nc.tensor.matmul(
                    ps[m][:],
                    yk[k][:, m * 128:(m + 1) * 128],
                    ahh[:],
                    start=(k == 0),
                    stop=(k == KT - 1),
                )
        for m in range(4):
            nc.scalar.activation(
                h1t[m][:, n * 512:(n + 1) * 512], ps[m][:],
                AFT.Relu, bias=b1_t[m][:], scale=1.0 / SCALE,
            )
            nc.vector.tensor_copy(h1h[m][:, n * 512:(n + 1) * 512],
                                  h1t[m][:, n * 512:(n + 1) * 512])

    # ---------------- heads (h1 half) + routing select -------------------
    for m in range(8):
        for k in range(4):
            nc.tensor.matmul(
                hreg(m, 0, E),
                h1t[k][:, m * 128:(m + 1) * 128],
                wp_t[k][:],
                start=(k == 0), stop=(k == 3),
            )
        for k in range(4):
            nc.tensor.matmul(
                hreg(m, 16, 16 + E * C),
                h1h[k][:, m * 128:(m + 1) * 128],
                wpp_t[k][:],
                start=(k == 0), stop=(k == 3),
            )
        hfin = small.tile([128, 240], F32, name="hfin")
        nc.vector.tensor_tensor(hfin[:], hreg(m, 0, 240), hpart[m][:],
                                op=ALU.add)
        sc = hfin[:, 0:E]
        oa = hfin[:, 16:16 + E * C]
        rmax = small.tile([128, 1], F32, name="rmax")
        nc.vector.tensor_reduce(rmax[:], sc, axis=mybir.AxisListType.X, op=ALU.max)
        val = small.tile([128, E], F32, name="val")
        nc.vector.tensor_scalar(val[:], sc, rmax[:], 1024.0, ALU.is_lt, ALU.mult)
        nc.vector.tensor_tensor(val[:], val[:], iota_t[:], op=ALU.add)
        idxf = small.tile([128, 1], F32, name="idxf")
        nc.vector.tensor_reduce(idxf[:], val[:], axis=mybir.AxisListType.X, op=ALU.min)
        onehot = small.tile([128, E], F32, name="onehot")
        nc.vector.tensor_scalar(onehot[:], val[:], idxf[:], None, ALU.is_equal)
        masked = small.tile([128, E, C], F32, name="masked")
        oa_v = oa.rearrange("p (e c) -> p e c", e=E)
        oh_v = onehot[:, :, None].broadcast_to((128, E, C))
        nc.vector.tensor_tensor(masked[:], oa_v, oh_v, op=ALU.mult)
        out_m = small.tile([128, C], F32, name="out_m")
        mv = masked[:].rearrange("p e c -> p c e")
        nc.vector.tensor_reduce(out_m[:], mv, axis=mybir.AxisListType.X, op=ALU.add)
        nc.sync.dma_start(out[m * 128:(m + 1) * 128, :], out_m[:])


def _build_nc():
    nc = bacc.Bacc("TRN2", target_bir_lowering=False, debug=False,
                   num_devices=NCORES)
    aps = {}
    def inp(name, shape, dt):
        aps[name] = nc.dram_tensor(name, shape, dt, kind="ExternalInput").ap()
    inp("F_cat", [N, 2 * IN], F16)
    inp("A_h", [N, BLK], F16)
    inp("A_l", [N, BLK], F16)
    inp("W0", [IN, H], F32)
    inp("W1f", [H, H], F32)
    inp("b0", [H, 1], F32)
    inp("b1", [H, 1], F32)
    inp("Wp32", [2 * H, E], F32)
    inp("Wpp16", [2 * H, E * C], F16)
    inp("iota7", [128, E], F32)
    aps["out"] = nc.dram_tensor("out", [BLK, C], F32, kind="ExternalOutput").ap()
    aps["cc_in"] = nc.dram_tensor("cc_in", [BLK, H], F16).ap()
    aps["cc_out"] = nc.dram_tensor("cc_out", [N, H], F16,
                                   addr_space="Shared").ap()
    from contextlib import ExitStack
    with tile.TileContext(nc) as tc, ExitStack() as ctx:
        _kernel_body(ctx, tc, aps)
    nc.compile()
    return nc


def _split16(x):
    h = x.astype(np.float16)
    l = (x - h.astype(np.float32)).astype(np.float16)
    return h, l


def kernel(feature, adj, W0, b0, W1, b1, Wp, Wpp):
    global LAST_RESULTS, _CACHED_NC
    feature = np.ascontiguousarray(np.asarray(feature, dtype=np.float32))
    adj = np.asarray(adj, dtype=np.float32)
    W0 = np.asarray(W0, dtype=np.float32)
    b0 = np.asarray(b0, dtype=np.float32)
    W1 = np.asarray(W1, dtype=np.float32)
    b1 = np.asarray(b1, dtype=np.float32)
    Wp = np.asarray(Wp, dtype=np.float32)
    Wpp = np.asarray(Wpp, dtype=np.float32)

    if _CACHED_NC is None:
        _CACHED_NC = _build_nc()
    nc = _CACHED_NC

    F_h, F_l = _split16(feature)
    F_cat = np.ascontiguousarray(np.concatenate([F_h, F_l], axis=1))
    Wpp16 = np.ascontiguousarray(
        Wpp.transpose(1, 0, 2).reshape(2 * H, E * C).astype(np.float16))
    iota7 = np.tile(np.arange(E, dtype=np.float32), (128, 1))
    shared = {
        "F_cat": F_cat,
        "W0": np.ascontiguousarray(W0),
        "W1f": np.ascontiguousarray(W1),
        "b0": b0.reshape(H, 1), "b1": b1.reshape(H, 1),
        "Wp32": np.ascontiguousarray(Wp),
        "Wpp16": Wpp16, "iota7": iota7,
    }
    in_maps = []
    for c in range(NCORES):
        blk = adj[c * BLK:(c + 1) * BLK, :].T.astype(np.float32) * SCALE
        A_h, A_l = _split16(blk)
        m = dict(shared)
        m["A_h"] = np.ascontiguousarray(A_h)
        m["A_l"] = np.ascontiguousarray(A_l)
        in_maps.append(m)

    trace = os.environ.get("BASS_KERNEL_TRACE", "0") == "1"
    res = run_bass_kernel_spmd(nc, in_maps, list(range(NCORES)), trace=trace)
    LAST_RESULTS = res
    out = np.concatenate([res.results[c]["out"] for c in range(NCORES)], axis=0)
    return out


# revision 9
# speedup vs baseline: 1.1642x; 1.0133x over previous
"""UNTESTED v7 candidate: v6 + gpsimd-issued Y loads after the collective.

Row-shards N=8192 nodes across 8 cores (1024 rows each). Architecture
per core (validated by end-to-end precision simulation on the exact
harness inputs):

  L0 (sensitive path):  TT = F^T @ adjT_blk  -- 3-pass fp16 hi/lo split
  h0 = relu(W0^T @ TT * 2^-13 + b0)          -- fp32 matmuls
  Y1 = h0 @ W1                               -- fp32 matmuls (score margin)
  AllGather(Y1-hi only, 1MB/rank)
  L1:  h1 = relu(Yh^T @ adjT-hi * 2^-13 + b1) -- 1-PASS fp16 (tolerant:
       score error via this path is ~1e-8 after cross-expert cancellation)
  scores = hc @ Wp                           -- exact fp32 matmuls
  out_all = hc16 @ Wpp16                     -- 1-pass fp16 (values only)
  one-hot argmax select                      -- as baseline

Precision: routing argmax has min top-2 gap 1.05e-7 on this input; only
the L0->h0 path needs fp32-grade accuracy (3-pass + fp32 h0 + fp32
scores). Simulated end-to-end: 0 routing flips, rel err ~1.2e-6.

Schedule: single AllGather of Yh, partially hidden by the h0-half of the
heads matmuls; L1 runs as two n-half sweeps (banks 0-3) with Y k-tiles
resident in SBUF after sweep A; heads partials accumulate in banks 4-7
across the L1 phase.
"""

import os
import numpy as np

import concourse.bass as bass
import concourse.mybir as mybir
import concourse.tile as tile
from concourse import bacc
from concourse.bass_utils import run_bass_kernel_spmd

N = 8192
IN = 512
H = 512
C = 32
E = 7
NCORES = 8
BLK = N // NCORES          # 1024 nodes per core
KT = N // 128              # 64 contraction k-tiles over nodes
SCALE = 8192.0             # adj pre-scale (exact power of two)
NW = E + E * C             # 231

F32 = mybir.dt.float32
F16 = mybir.dt.float16

LAST_RESULTS = None
_CACHED_NC = None


def _kernel_body(ctx, tc, aps):
    nc = tc.nc
    AFT = mybir.ActivationFunctionType
    ALU = mybir.AluOpType

    F_cat = aps["F_cat"]
    A_h, A_l = aps["A_h"], aps["A_l"]
    W0, W1f = aps["W0"], aps["W1f"]
    b0, b1 = aps["b0"], aps["b1"]
    Wp32, Wpp16 = aps["Wp32"], aps["Wpp16"]
    iota7 = aps["iota7"]
    out = aps["out"]
    cc_in, cc_out = aps["cc_in"], aps["cc_out"]

    const = ctx.enter_context(tc.tile_pool(name="const", bufs=1))
    acts = ctx.enter_context(tc.tile_pool(name="acts", bufs=1))
    stream = ctx.enter_context(tc.tile_pool(name="stream", bufs=4))
    ypool = ctx.enter_context(tc.tile_pool(name="ypool", bufs=1))
    small = ctx.enter_context(tc.tile_pool(name="small", bufs=4))
    psum = ctx.enter_context(tc.tile_pool(name="psum", bufs=1, space="PSUM"))

    ps = [psum.tile([128, 512], F32, name=f"bank{i}") for i in range(8)]

    # ---------- L0 stream prefetch for first tiles, then consts ----------
    l0_tiles = []

    def l0_fetch(k):
        fc = stream.tile([128, 1024], F16, name="fc")
        ah = stream.tile([128, 1024], F16, name="ah")
        al = stream.tile([128, 1024], F16, name="al")
        r = slice(k * 128, (k + 1) * 128)
        nc.sync.dma_start(fc[:], F_cat[r, :])
        nc.sync.dma_start(ah[:], A_h[r, :])
        nc.sync.dma_start(al[:], A_l[r, :])
        return fc, ah, al

    # prefetch k=0..2 before const weights so the PE starts ASAP and the
    # stream stays ahead of the ~6.3us/tile PE consumption rate; slot the
    # consts (needed only ~400us in) behind the early stream tiles
    for k in range(3):
        l0_tiles.append(l0_fetch(k))

    w0_t, w1_t, wp_t, wpp_t = [], [], [], []
    for k in range(4):
        t = const.tile([128, H], F32, name=f"w0_{k}")
        nc.sync.dma_start(t[:], W0[k * 128:(k + 1) * 128, :])
        w0_t.append(t)
        t = const.tile([128, H], F32, name=f"w1_{k}")
        nc.sync.dma_start(t[:], W1f[k * 128:(k + 1) * 128, :])
        w1_t.append(t)
    l0_tiles.append(l0_fetch(3))
    for k in range(8):
        t = const.tile([128, E], F32, name=f"wp_{k}")
        nc.sync.dma_start(t[:], Wp32[k * 128:(k + 1) * 128, :])
        wp_t.append(t)
        t = const.tile([128, E * C], F16, name=f"wpp_{k}")
        nc.sync.dma_start(t[:], Wpp16[k * 128:(k + 1) * 128, :])
        wpp_t.append(t)
    b0_t, b1_t = [], []
    for m in range(4):
        t = const.tile([128, 1], F32, name=f"b0_{m}")
        nc.sync.dma_start(t[:], b0[m * 128:(m + 1) * 128, :])
        b0_t.append(t)
        t = const.tile([128, 1], F32, name=f"b1_{m}")
        nc.sync.dma_start(t[:], b1[m * 128:(m + 1) * 128, :])
        b1_t.append(t)
    iota_t = const.tile([128, E], F32, name="iota7")
    nc.sync.dma_start(iota_t[:], iota7[:, :])

    # ---------------- L0: TT[m,n] = sum_k F[k][:,m].T @ A[k][:,n] --------
    for k in range(KT):
        fc, ah, al = l0_tiles[k]
        if k + 4 < KT:
            l0_tiles.append(l0_fetch(k + 4))
        for p, (lo, rt) in enumerate(((0, ah), (512, ah), (0, al))):
            for m in range(4):
                for n in range(2):
                    nc.tensor.matmul(
                        ps[m * 2 + n][:],
                        fc[:, lo + m * 128:lo + (m + 1) * 128],
                        rt[:, n * 512:(n + 1) * 512],
                        start=(k == 0 and p == 0),
                        stop=(k == KT - 1 and p == 2),
                    )

    # ---------------- TT drain, h0 (fp32), h0h cast ----------------------
    tt = [acts.tile([128, BLK], F32, name=f"tt_{m}") for m in range(4)]
    for n in range(2):
        for m in range(4):
            nc.vector.tensor_copy(tt[m][:, n * 512:(n + 1) * 512],
                                  ps[m * 2 + n][:])

    h0t = [acts.tile([128, BLK], F32, name=f"h0t_{m}") for m in range(4)]
    h0h = [acts.tile([128, BLK], F16, name=f"h0h_{m}") for m in range(4)]
    for n in range(2):
        for m in range(4):
            pt = ps[m * 2 + n]
            for k in range(4):
                nc.tensor.matmul(
                    pt[:],
                    w0_t[k][:, m * 128:(m + 1) * 128],
                    tt[k][:, n * 512:(n + 1) * 512],
                    start=(k == 0),
                    stop=(k == 3),
                )
            nc.scalar.activation(
                h0t[m][:, n * 512:(n + 1) * 512], pt[:],
                AFT.Relu, bias=b0_t[m][:], scale=1.0 / SCALE,
            )
            nc.vector.tensor_copy(h0h[m][:, n * 512:(n + 1) * 512],
                                  h0t[m][:, n * 512:(n + 1) * 512])

    # ---------------- Y1 = h0 @ W1, 1-pass fp16; gather Yh --------------
    yh = [ypool.tile([128, H], F16, name=f"yh_{m}") for m in range(8)]
    for m in range(8):
        pt = ps[m]
        for k in range(4):
            nc.tensor.matmul(
                pt[:],
                h0t[k][:, m * 128:(m + 1) * 128],
                w1_t[k][:],
                start=(k == 0),
                stop=(k == 3),
            )
        nc.vector.tensor_copy(yh[m][:], pt[:])
        nc.sync.dma_start(cc_in[m * 128:(m + 1) * 128, :], yh[m][:])

    nc.gpsimd.collective_compute(
        "AllGather",
        mybir.AluOpType.bypass,
        replica_groups=[list(range(NCORES))],
        ins=[cc_in.opt()],
        outs=[cc_out.opt()],
    )

    # ---------------- heads partials (h0 half) while AG runs -------------
    # heads region for node-block m lives in bank 4 + m//2, col 256*(m%2);
    # scores at cols 0:7, out_all at cols 16:240 (aligned)
    def hreg(m, c0, c1):
        return ps[4 + m // 2][:, 256 * (m % 2) + c0:256 * (m % 2) + c1]

    # interleave m across banks (bank = 4 + m//2) so one block's PSUM
    # drain never stalls the next block's matmuls (tile-granular deps)
    M_ORDER = (0, 2, 4, 6, 1, 3, 5, 7)

    hpart = [acts.tile([128, 240], F32, name=f"hpart_{m}") for m in range(8)]
    for m in M_ORDER:
        for k in range(4):
            nc.tensor.matmul(
                hreg(m, 0, E),
                h0t[k][:, m * 128:(m + 1) * 128],
                wp_t[4 + k][:],
                start=(k == 0), stop=(k == 3),
            )
        for k in range(4):
            nc.tensor.matmul(
                hreg(m, 16, 16 + E * C),
                h0h[k][:, m * 128:(m + 1) * 128],
                wpp_t[4 + k][:],
                start=(k == 0), stop=(k == 3),
            )
        nc.vector.tensor_copy(hpart[m][:], hreg(m, 0, 240))

    # ---------------- L1: two n-half sweeps, banks 0-3 ------------------
    yk = [ypool.tile([128, H], F16, name=f"yk_{k}") for k in range(KT)]
    h1t = [acts.tile([128, BLK], F32, name=f"h1t_{m}") for m in range(4)]
    h1h = [acts.tile([128, BLK], F16, name=f"h1h_{m}") for m in range(4)]

    for n in range(2):
        for k in range(KT):
            r = slice(k * 128, (k + 1) * 128)
            if n == 0:
                # issue the AG-gated Y loads from the gpsimd engine (the
                # collective's own engine): they cannot enter the SDMA
                # queues until the collective retires, so they never starve
                # its transfers (the 43us-vs-84us AG variance across builds)
                nc.gpsimd.dma_start(yk[k][:], cc_out[r, :])
            ahh = stream.tile([128, 512], F16, name="ahh")
            nc.sync.dma_start(ahh[:], A_h[r, n * 512:(n + 1) * 512])
            for m in range(4):
                nc.tensor.matmul(
                    ps[m][:],
                    yk[k][:, m * 128:(m + 1) * 128],
                    ahh[:],
                    start=(k == 0),
                    stop=(k == KT - 1),
                )
        for m in range(4):
            nc.scalar.activation(
                h1t[m][:, n * 512:(n + 1) * 512], ps[m][:],
                AFT.Relu, bias=b1_t[m][:], scale=1.0 / SCALE,
            )
            nc.vector.tensor_copy(h1h[m][:, n * 512:(n + 1) * 512],
                                  h1t[m][:, n * 512:(n + 1) * 512])

    # ---------------- heads (h1 half) + routing select -------------------
    for m in M_ORDER:
        for k in range(4):
            nc.tensor.matmul(
                hreg(m, 0, E),
                h1t[k][:, m * 128:(m + 1) * 128],
                wp_t[k][:],
                start=(k == 0), stop=(k == 3),
            )
        for k in range(4):
            nc.tensor.matmul(
                hreg(m, 16, 16 + E * C),
                h1h[k][:, m * 128:(m + 1) * 128],
                wpp_t[k][:],
                start=(k == 0), stop=(k == 3),
            )
        hfin = small.tile([128, 240], F32, name="hfin")
        nc.vector.tensor_tensor(hfin[:], hreg(m, 0, 240), hpart[m][:],
                                op=ALU.add)
        sc = hfin[:, 0:E]
        oa = hfin[:, 16:16 + E * C]
        rmax = small.tile([128, 1], F32, name="rmax")
        nc.vector.tensor_reduce(rmax[:], sc, axis=mybir.AxisListType.X, op=ALU.max)
        val = small.tile([128, E], F32, name="val")
        nc.vector.tensor_scalar(val[:], sc, rmax[:], 1024.0, ALU.is_lt, ALU.mult)
        nc.vector.tensor_tensor(val[:], val[:], iota_t[:], op=ALU.add)
        idxf = small.tile([128, 1], F32, name="idxf")
        nc.vector.tensor_reduce(idxf[:], val[:], axis=mybir.AxisListType.X, op=ALU.min)
        onehot = small.tile([128, E], F32, name="onehot")
        nc.vector.tensor_scalar(onehot[:], val[:], idxf[:], None, ALU.is_equal)
        masked = small.tile([128, E, C], F32, name="masked")
        oa_v = oa.rearrange("p (e c) -> p e c", e=E)
        oh_v = onehot[:, :, None].broadcast_to((128, E, C))
        nc.vector.tensor_tensor(masked[:], oa_v, oh_v, op=ALU.mult)
        out_m = small.tile([128, C], F32, name="out_m")
        mv = masked[:].rearrange("p e c -> p c e")
        nc.vector.tensor_reduce(out_m[:], mv, axis=mybir.AxisListType.X, op=ALU.add)
        nc.sync.dma_start(out[m * 128:(m + 1) * 128, :], out_m[:])


def _build_nc():
    nc = bacc.Bacc("TRN2", target_bir_lowering=False, debug=False,
                   num_devices=NCORES)
    aps = {}
    def inp(name, shape, dt):
        aps[name] = nc.dram_tensor(name, shape, dt, kind="ExternalInput").ap()
    inp("F_cat", [N, 2 * IN], F16)
    inp("A_h", [N, BLK], F16)
    inp("A_l", [N, BLK], F16)
    inp("W0", [IN, H], F32)
    inp("W1f", [H, H], F32)
    inp("b0", [H, 1], F32)
    inp("b1", [H, 1], F32)
    inp("Wp32", [2 * H, E], F32)
    inp("Wpp16", [2 * H, E * C], F16)
    inp("iota7", [128, E], F32)
    aps["out"] = nc.dram_tensor("out", [BLK, C], F32, kind="ExternalOutput").ap()
    aps["cc_in"] = nc.dram_tensor("cc_in", [BLK, H], F16).ap()
    aps["cc_out"] = nc.dram_tensor("cc_out", [N, H], F16,
                                   addr_space="Shared").ap()
    from contextlib import ExitStack
    with tile.TileContext(nc) as tc, ExitStack() as ctx:
        _kernel_body(ctx, tc, aps)
    nc.compile()
    return nc


def _split16(x):
    h = x.astype(np.float16)
    l = (x - h.astype(np.float32)).astype(np.float16)
    return h, l


def kernel(feature, adj, W0, b0, W1, b1, Wp, Wpp):
    global LAST_RESULTS, _CACHED_NC
    feature = np.ascontiguousarray(np.asarray(feature, dtype=np.float32))
    adj = np.asarray(adj, dtype=np.float32)
    W0 = np.asarray(W0, dtype=np.float32)
    b0 = np.asarray(b0, dtype=np.float32)
    W1 = np.asarray(W1, dtype=np.float32)
    b1 = np.asarray(b1, dtype=np.float32)
    Wp = np.asarray(Wp, dtype=np.float32)
    Wpp = np.asarray(Wpp, dtype=np.float32)

    if _CACHED_NC is None:
        _CACHED_NC = _build_nc()
    nc = _CACHED_NC

    F_h, F_l = _split16(feature)
    F_cat = np.ascontiguousarray(np.concatenate([F_h, F_l], axis=1))
    Wpp16 = np.ascontiguousarray(
        Wpp.transpose(1, 0, 2).reshape(2 * H, E * C).astype(np.float16))
    iota7 = np.tile(np.arange(E, dtype=np.float32), (128, 1))
    shared = {
        "F_cat": F_cat,
        "W0": np.ascontiguousarray(W0),
        "W1f": np.ascontiguousarray(W1),
        "b0": b0.reshape(H, 1), "b1": b1.reshape(H, 1),
        "Wp32": np.ascontiguousarray(Wp),
        "Wpp16": Wpp16, "iota7": iota7,
    }
    in_maps = []
    for c in range(NCORES):
        blk = adj[c * BLK:(c + 1) * BLK, :].T.astype(np.float32) * SCALE
        A_h, A_l = _split16(blk)
        m = dict(shared)
        m["A_h"] = np.ascontiguousarray(A_h)
        m["A_l"] = np.ascontiguousarray(A_l)
        in_maps.append(m)

    trace = os.environ.get("BASS_KERNEL_TRACE", "0") == "1"
    res = run_bass_kernel_spmd(nc, in_maps, list(range(NCORES)), trace=trace)
    LAST_RESULTS = res
    out = np.concatenate([res.results[c]["out"] for c in range(NCORES)], axis=0)
    return out
